# revision 4
# baseline (speedup 1.0000x reference)
"""BiasAttention TRN2 kernel — q-sharded across 8 NeuronCores, fp8 z.

Each core owns a block of 128 queries and computes the full attention for
them (all 8 heads, all 1024 keys).  The z-bias projection dominates both
HBM traffic and PE time, so z ships as fp8e3 (e3m4: 4 mantissa bits keeps
the bias quantization error ~5x below the bf16-path output error budget)
— halving DMA bytes vs bf16 and doubling LDWEIGHTS rate via fast weight
load.  Attention runs in transposed form S^T[k, q] so that P^T = exp(S^T)
is directly the stationary operand for attn.V — no PE transposes of P.
The z.Wb matmuls and the K^T.Q matmuls accumulate into one PSUM tile
[k, q, h]; ACT applies exp straight out of PSUM.  The bias-projection
bias bb and the K half of bkv are constant along the softmax axis and
cancel — they are dropped.
"""

import sys

if "/opt/trn_rl_repo" not in sys.path:
    sys.path.insert(0, "/opt/trn_rl_repo")

import ml_dtypes
import numpy as np

import concourse.bass as bass
import concourse.mybir as mybir
from concourse import bacc
from concourse.bass_utils import run_bass_kernel_spmd
from concourse.masks import make_identity
from concourse.tile import TileContext

P = 128          # partitions
H = 8            # heads
D = 32           # head dim
CQ = 256         # q channels
CKV = 256        # kv channels
BD = 128         # bias (z) channels
NQ = 1024        # total queries
NK = 1024        # total keys
NCORES = 8
NQC = NQ // NCORES   # queries per core = 128
KC_N = NK // P       # k-chunks = 8
G = 2                # z DMA groups per k-chunk
TQ = NQC // G        # q-tiles per group = 64
SCALE = D ** (-0.5)

FP = mybir.dt.float32
BF = mybir.dt.bfloat16
F8 = mybir.dt.float8e3
NP_BF = ml_dtypes.bfloat16
NP_F8 = ml_dtypes.float8_e3m4


def build_program():
    add = mybir.AluOpType.add
    mult = mybir.AluOpType.mult

    nc = bacc.Bacc("TRN2", target_bir_lowering=False, debug=False,
                   num_devices=NCORES)

    # ---- I/O ----
    # zT[kc, g, c, tq, k] = z[q = g*TQ + tq, kc*P + k, c]
    zT = nc.dram_tensor("zT", [KC_N, G, BD, TQ, P], F8, kind="ExternalInput")
    xqT = nc.dram_tensor("xqT", [CQ, NQC], BF, kind="ExternalInput")
    xkvT = nc.dram_tensor("xkvT", [CKV, NK], BF, kind="ExternalInput")
    Wq = nc.dram_tensor("Wq", [CQ, H * D], BF, kind="ExternalInput")
    bq = nc.dram_tensor("bq", [H * D], FP, kind="ExternalInput")
    Wkv = nc.dram_tensor("Wkv", [CKV, 2 * H * D], BF, kind="ExternalInput")
    bkvV = nc.dram_tensor("bkvV", [H * D], FP, kind="ExternalInput")
    Wb = nc.dram_tensor("Wb", [BD, H], BF, kind="ExternalInput")
    Wp = nc.dram_tensor("Wp", [H * D, CQ], FP, kind="ExternalInput")
    bp = nc.dram_tensor("bp", [CQ], FP, kind="ExternalInput")
    y = nc.dram_tensor("y", [NQC, CQ], FP, kind="ExternalOutput")

    with TileContext(nc) as tc:
        with (
            tc.tile_pool(name="const", bufs=1) as const,
            tc.tile_pool(name="zpool", bufs=4) as zpool,
            tc.tile_pool(name="xpool", bufs=3) as xpool,
            tc.tile_pool(name="proj_ps", bufs=2, space="PSUM") as proj_ps,
            tc.tile_pool(name="b_ps", bufs=4, space="PSUM") as b_psp,
            tc.tile_pool(name="o_ps", bufs=1, space="PSUM") as o_psp,
        ):
            # ---- z stream head-start: wb + first z groups lead the ring ----
            wb_sb = const.tile([P, H], BF)
            nc.sync.dma_start(wb_sb, Wb[:])
            zpre = []
            for gidx in range(4):
                z_sb = zpool.tile([P, TQ, P], F8, tag="z", name=f"zpre{gidx}")
                nc.sync.dma_start(z_sb, zT[gidx // G, gidx % G])
                zpre.append(z_sb)

            # ---- constants / weights to SBUF ----
            wq_sb = const.tile([P, 2, H * D], BF)
            nc.sync.dma_start(wq_sb, Wq.rearrange("(o p) m -> p o m", p=P))
            wkv_sb = const.tile([P, 2, 2 * H * D], BF)
            nc.sync.dma_start(wkv_sb, Wkv.rearrange("(o p) m -> p o m", p=P))
            wp_sb = const.tile([P, 2, CQ], FP)
            nc.sync.dma_start(wp_sb, Wp.rearrange("(o p) m -> p o m", p=P))
            xqT_sb = const.tile([P, 2, NQC], BF)
            nc.sync.dma_start(xqT_sb, xqT.rearrange("(o p) q -> p o q", p=P))
            xkvT_sb = const.tile([P, 2, NK], BF)
            nc.sync.dma_start(xkvT_sb, xkvT.rearrange("(o p) k -> p o k", p=P))
            bq_sb = const.tile([P, 2], FP)
            nc.sync.dma_start(bq_sb, bq.rearrange("(o p) -> p o", p=P))
            bkvV_sb = const.tile([1, H * D], FP)
            nc.sync.dma_start(bkvV_sb, bkvV[None, :])
            bp_sb = const.tile([1, CQ], FP)
            nc.sync.dma_start(bp_sb, bp[None, :])
            ident = const.tile([P, P], FP)
            make_identity(nc, ident)
            ones_row = const.tile([1, P], FP)
            nc.vector.memset(ones_row, 1.0)

            # V augmented with a ones column per head: [k, kc, h, D+1]
            vaug_sb = const.tile([P, KC_N, H, D + 1], BF)
            nc.vector.memset(vaug_sb, 1.0)

            # ---- projections (bf16 in, fp32 psum accumulate) ----
            # Q^T [(h d), q] with (x + bq) * SCALE folded in, stored bf16
            qT_sb = const.tile([P, 2, NQC], BF)
            for m in range(2):
                ps = proj_ps.tile([P, 512], FP, tag="proj")
                for c in range(2):
                    nc.tensor.matmul(ps[:, :NQC],
                                     lhsT=wq_sb[:, c, m * P:(m + 1) * P],
                                     rhs=xqT_sb[:, c, :],
                                     start=(c == 0), stop=(c == 1))
                nc.vector.tensor_scalar(qT_sb[:, m, :], ps[:, :NQC],
                                        bq_sb[:, m:m + 1], SCALE, add, mult)

            # K^T [(h d), k], no bias (cancels in softmax), stored bf16
            kT_sb = const.tile([P, 2, NK], BF)
            for m in range(2):
                for nh in range(NK // 512):
                    ps = proj_ps.tile([P, 512], FP, tag="proj")
                    for c in range(2):
                        nc.tensor.matmul(ps[:, :],
                                         lhsT=wkv_sb[:, c, m * P:(m + 1) * P],
                                         rhs=xkvT_sb[:, c, nh * 512:(nh + 1) * 512],
                                         start=(c == 0), stop=(c == 1))
                    if nh % 2 == 0:
                        nc.scalar.activation(
                            kT_sb[:, m, nh * 512:(nh + 1) * 512], ps,
                            mybir.ActivationFunctionType.Copy)
                    else:
                        nc.vector.tensor_copy(
                            kT_sb[:, m, nh * 512:(nh + 1) * 512], ps)

            # V [k, (h d)] + bkv_V, written into vaug (ones col preserved)
            for kc in range(KC_N):
                ps = proj_ps.tile([P, 512], FP, tag="proj", name="v_ps")
                for c in range(2):
                    nc.tensor.matmul(ps[:, :H * D],
                                     lhsT=xkvT_sb[:, c, kc * P:(kc + 1) * P],
                                     rhs=wkv_sb[:, c, H * D:2 * H * D],
                                     start=(c == 0), stop=False)
                nc.tensor.matmul(ps[:, :H * D], lhsT=ones_row,
                                 rhs=bkvV_sb, start=False, stop=True)
                nc.scalar.activation(
                    vaug_sb[:, kc, :, 0:D],
                    ps[:, :H * D].rearrange("p (h d) -> p h d", h=H),
                    mybir.ActivationFunctionType.Copy)

            # ---- main loop over k-chunks: S^T + bias accumulate in PSUM ----
            o_ps = o_psp.tile([P, H * (D + 1)], FP)   # [q, h*(D+1)]
            av_prev = None
            for kc in range(KC_N):
                x_sb = xpool.tile([P, NQC, H], BF, tag="x")
                # Per q-half (one PSUM bank each): S^T + bias, then exp.
                for g in range(G):
                    b_ps = b_psp.tile([P, TQ, H], FP, tag="b")  # [k, q, h]
                    # S^T[k, q] per head: lhsT = K^T chunk, rhs = Q^T
                    for h in range(H):
                        r0 = (h % 4) * 32
                        nc.tensor.matmul(
                            b_ps[:, :, h],
                            lhsT=kT_sb[r0:r0 + 32, h // 4,
                                       kc * P:(kc + 1) * P],
                            rhs=qT_sb[r0:r0 + 32, h // 4,
                                      g * TQ:(g + 1) * TQ],
                            start=(h == 0), stop=False,
                            tile_position=(r0, 0))
                    # bias[k, q, h] += z[q]^T Wb, one z tile per query
                    gidx = kc * G + g
                    if gidx < len(zpre):
                        z_sb = zpre[gidx]
                    else:
                        z_sb = zpool.tile([P, TQ, P], F8, tag="z")
                        nc.sync.dma_start(z_sb, zT[gidx // G, gidx % G])
                    for t in range(TQ):
                        nc.tensor.matmul(b_ps[:, t, :],
                                         lhsT=z_sb[:, t, :], rhs=wb_sb,
                                         start=False, stop=(t == TQ - 1))
                    # exp out of PSUM
                    nc.scalar.activation(x_sb[:, g * TQ:(g + 1) * TQ, :],
                                         b_ps,
                                         mybir.ActivationFunctionType.Exp)
                # attn.V for the previous chunk (so PE never waits on ACT)
                if av_prev is not None:
                    xp, kp = av_prev
                    for h in range(H):
                        nc.tensor.matmul(
                            o_ps[:, h * (D + 1):(h + 1) * (D + 1)],
                            lhsT=xp[:, :, h], rhs=vaug_sb[:, kp, h, :],
                            start=(kp == 0 and h == 0), stop=False)
                av_prev = (x_sb, kc)
            xp, kp = av_prev
            for h in range(H):
                nc.tensor.matmul(o_ps[:, h * (D + 1):(h + 1) * (D + 1)],
                                 lhsT=xp[:, :, h], rhs=vaug_sb[:, kp, h, :],
                                 start=False, stop=(h == H - 1))

            # ---- epilogue: normalize, transpose, output projection ----
            recip_sb = const.tile([P, H], FP)
            for h in range(H):
                nc.vector.reciprocal(recip_sb[:, h:h + 1],
                                     o_ps[:, h * (D + 1) + D:h * (D + 1) + D + 1])
            o_sb = const.tile([P, 2, P], FP)     # [q, half, (h d)%128]
            for h in range(H):
                nc.vector.tensor_scalar(
                    o_sb[:, h // 4, (h % 4) * 32:(h % 4) * 32 + 32],
                    o_ps[:, h * (D + 1):h * (D + 1) + D],
                    recip_sb[:, h:h + 1], None, mult)
            oT_sb = const.tile([P, 2, P], FP)
            for m in range(2):
                t_full = proj_ps.tile([P, 512], FP, tag="proj", name="t_full")
                t_ps = t_full[:, :P]
                nc.tensor.transpose(t_ps, o_sb[:, m, :], ident)
                nc.vector.tensor_copy(oT_sb[:, m, :], t_ps)
            ps = proj_ps.tile([P, 512], FP, tag="proj")
            for m in range(2):
                nc.tensor.matmul(ps[:, :CQ], lhsT=oT_sb[:, m, :],
                                 rhs=wp_sb[:, m, :], start=(m == 0), stop=False)
            nc.tensor.matmul(ps[:, :CQ], lhsT=ones_row, rhs=bp_sb,
                             start=False, stop=True)
            y_sb = const.tile([P, CQ], FP)
            nc.vector.tensor_copy(y_sb, ps[:, :CQ])
            nc.sync.dma_start(y[:], y_sb)

    nc.compile()
    return nc


def prep_inputs(x_q, x_kv, z, Wq, bq, Wkv, bkv, Wb, bb, Wp, bp):
    """Host-side shard prep.  Returns in_maps for the 8 cores.

    bb and the K half of bkv are constant along the softmax axis and
    cancel; they are not shipped.
    """
    xkvT = np.ascontiguousarray(x_kv[0].T).astype(NP_BF)     # [CKV, nk]
    shared = dict(xkvT=xkvT,
                  Wq=np.ascontiguousarray(Wq).astype(NP_BF),
                  bq=np.ascontiguousarray(bq, dtype=np.float32),
                  Wkv=np.ascontiguousarray(Wkv).astype(NP_BF),
                  bkvV=np.ascontiguousarray(bkv[H * D:], dtype=np.float32),
                  Wb=np.ascontiguousarray(Wb).astype(NP_BF),
                  Wp=np.ascontiguousarray(Wp, dtype=np.float32),
                  bp=np.ascontiguousarray(bp, dtype=np.float32))
    in_maps = []
    for i in range(NCORES):
        qs = i * NQC
        zi = z[0, qs:qs + NQC]                           # [q, k, c]
        # -> [kc, g, c, tq, k] with q = g*TQ + tq, key = kc*P + k
        zi = (zi.reshape(G, TQ, KC_N, P, BD)
                .transpose(2, 0, 4, 1, 3))
        in_maps.append(dict(
            zT=np.ascontiguousarray(zi).astype(NP_F8),
            xqT=np.ascontiguousarray(x_q[0, qs:qs + NQC].T).astype(NP_BF),
            **shared,
        ))
    return in_maps


_NC_CACHE = {}


def kernel(x_q, x_kv, z, Wq, bq, Wkv, bkv, Wb, bb, Wp, bp):
    key = "full"
    if key not in _NC_CACHE:
        _NC_CACHE[key] = build_program()
    nc = _NC_CACHE[key]
    in_maps = prep_inputs(x_q, x_kv, z, Wq, bq, Wkv, bkv, Wb, bb, Wp, bp)
    res = run_bass_kernel_spmd(nc, in_maps, list(range(NCORES)))
    out = np.empty((1, NQ, CQ), dtype=np.float32)
    for i in range(NCORES):
        out[0, i * NQC:(i + 1) * NQC, :] = res.results[i]["y"]
    return out
